# revision 9
# baseline (speedup 1.0000x reference)
"""Trainium2 Bass kernel for LocalSpatioTemporalPooling (topk masking).

Reference computation (per sample n):
  x: (N=16, C=256, T=30, H=64, W=32) f32
  ff[n,c,t,s]   = mean over the (8,32) stripe s of the (H,W) plane
  score[n,t,s]  = sum_c ff^2   (monotone in the reference's sqrt/clip score)
  top-2 t per (n,s) by score; output[n, s*256+c] = mean of ff over those 2 t.

Strategy: pure data parallel over batch N across 8 cores (2 samples/core).
Inputs are shipped to the device as fp16 (exact top-2 sets and ~2e-4 output
rel err for randn inputs -- verified against the f32 reference), which halves
both host->device traffic and the on-device HBM read volume.

Per core (62.9 MB fp16):
  phase 1 (memory bound): stream x in 20 tiles of 3 MB, partition = channel
    (128 channels per half ci), free = 6 frames x 2048 spatial.  GpSimd folds
    element pairs within each 256-element stripe segment (fp16 add), DVE then
    reduce_sums the folded 128-element segments into f32 stripe sums, laid
    out as ffq[ci][c_part, (n, t, s)] directly in SBUF -- no DRAM roundtrip.
  phase 2 (tiny): square + ones-matmul on PE -> per-(n,t,s) score; top-2 via
    mask = (score >= 2nd max) scaled by 1/512 (fuses the /2 top-k mean and
    the /256 stripe mean); broadcast mask over partitions via PE outer
    product; masked reduce over t -> out columns (ci, c_local, s), reordered
    on host.
"""

import sys
from concurrent.futures import ThreadPoolExecutor
from contextlib import ExitStack

for _p in ("/opt/trn_rl_repo",):
    if _p not in sys.path:
        sys.path.insert(0, _p)

import numpy as np

import concourse.bass as bass
import concourse.tile as tile
from concourse import bacc, mybir
from concourse.bass_utils import run_bass_kernel_spmd

N_CORES = 8
N, C, T, H, W = 16, 256, 30, 64, 32
S = 8            # stripes
E = (H // S) * W  # 256 elements per stripe segment
NL = N // N_CORES  # samples per core = 2
HW = H * W       # 2048
T_SUB = 6        # frames per tile
NT = T // T_SUB  # 5 t-chunks
TS = T * S       # 240
F2 = NL * TS     # 480
OUT_COLS = S * C  # 2048
F32 = mybir.dt.float32
F16 = mybir.dt.float16
X = mybir.AxisListType.X


def build_program() -> bacc.Bacc:
    nc = bacc.Bacc("TRN2", target_bir_lowering=False, debug=False,
                   num_devices=N_CORES)
    x = nc.dram_tensor("x", [NL, C, T, HW], F16, kind="ExternalInput").ap()
    out = nc.dram_tensor("out", [NL, OUT_COLS], F32, kind="ExternalOutput").ap()

    with tile.TileContext(nc) as tc, ExitStack() as ctx:
        xpool = ctx.enter_context(tc.tile_pool(name="xtiles", bufs=4))
        fpool = ctx.enter_context(tc.tile_pool(name="folds", bufs=3))
        cpool = ctx.enter_context(tc.tile_pool(name="consts", bufs=1))
        spool = ctx.enter_context(tc.tile_pool(name="small", bufs=1))
        ppool = ctx.enter_context(tc.tile_pool(name="psum", bufs=1, space="PSUM"))

        # ---- phase 1: per-stripe sums -> ffq[ci][c, (n, t, s)] in SBUF ----
        # Per tile: DVE folds the halves of each 256-el stripe segment (fp16
        # add; 128-el runs keep the 2x packed mode, ~3.4 us), GpSimd folds
        # once more (64-el runs, ~7.4 us -- GpSimd has no packing modes to
        # lose), DVE reduce_sums the 64-el segments into f32 (~3.3 us).  No
        # stage exceeds the ~9.6 us/tile DMA cadence -> DMA-bound stream.
        # The last TAIL_TILES tiles skip the GpSimd hop (DVE reduces the
        # once-folded tile directly) to shorten the post-DMA tail.
        add = mybir.AluOpType.add
        NTILES = NL * 2 * NT
        TAIL_TILES = 2
        ffq = [cpool.tile([128, NL, T, S], F32, name=f"ffq{ci}") for ci in range(2)]
        i = 0
        for ci in range(2):
            for n_ in range(NL):
                for tc_ in range(NT):
                    xt = xpool.tile([128, T_SUB * HW], F16, name="xt", tag="xt")
                    eng = nc.sync if (i % 2 == 0) else nc.scalar
                    eng.dma_start(
                        xt[:],
                        x[n_, ci * 128:(ci + 1) * 128,
                          tc_ * T_SUB:(tc_ + 1) * T_SUB, :]
                        .rearrange("c k m -> c (k m)"),
                    )
                    v4 = xt[:].rearrange("p (k s e) -> p k s e", k=T_SUB, s=S)
                    ft = fpool.tile([128, T_SUB, S, E // 2], F16,
                                    name="ft", tag="ft")
                    nc.vector.tensor_tensor(
                        ft[:], v4[:, :, :, 0:E // 2], v4[:, :, :, E // 2:E],
                        op=add,
                    )
                    out_slice = ffq[ci][:, n_, tc_ * T_SUB:(tc_ + 1) * T_SUB, :]
                    if i < NTILES - TAIL_TILES:
                        ft2 = fpool.tile([128, T_SUB, S, E // 4], F16,
                                         name="ft2", tag="ft2")
                        nc.gpsimd.tensor_tensor(
                            ft2[:], ft[:, :, :, 0:E // 4],
                            ft[:, :, :, E // 4:E // 2], op=add,
                        )
                        nc.vector.reduce_sum(out_slice, ft2[:], axis=X)
                    else:
                        nc.vector.reduce_sum(out_slice, ft[:], axis=X)
                    i += 1

        # ---- phase 2: scores, top-2 mask, masked mean ----
        ones_col = cpool.tile([128, 1], F32)   # K=128 stationary: column of ones
        nc.vector.memset(ones_col[:], 1.0)
        ones_row = cpool.tile([1, 128], F32)   # K=1 stationary: row of 1/512
        nc.vector.memset(ones_row[:], 1.0 / 512.0)

        psc = ppool.tile([1, F2], F32, name="psc", tag="psc")  # sum_c ff^2
        for ci in range(2):
            sq = spool.tile([128, F2], F32, name=f"sq{ci}", tag=f"sq{ci}")
            nc.scalar.square(sq[:], ffq[ci][:].rearrange("p a b c -> p (a b c)"))
            nc.tensor.matmul(psc[:], ones_col[:], sq[:],
                             start=(ci == 0), stop=(ci == 1))

        sc_sb = spool.tile([1, F2], F32, name="sc_sb")
        nc.scalar.copy(sc_sb[:], psc[:])

        # top-2 mask per (n, s) segment, computed in place on one partition.
        # seg: (q, n, s, t) view for per-(n,s) reduces over t;
        # v4/bc: matched 4D (q, n, t, s) iteration, bc has stride-0 over t.
        def seg(ap):
            return ap.rearrange("q (n t s) -> q n s t", n=NL, t=T, s=S)

        def v4(ap):
            return ap.rearrange("q (n t s) -> q n t s", n=NL, t=T, s=S)

        def bc(ap):
            return (ap.rearrange("q (n s) -> q n s", n=NL)[:, :, None, :]
                    .broadcast_to((1, NL, T, S)))

        ge = mybir.AluOpType.is_ge
        m1 = spool.tile([1, NL * S], F32, name="m1")
        nc.vector.reduce_max(m1[:], seg(sc_sb[:]), axis=X)
        eqb = spool.tile([1, F2], F32, name="eqb")
        nc.vector.tensor_tensor(v4(eqb[:]), v4(sc_sb[:]), bc(m1[:]), op=ge)
        nc.vector.tensor_scalar(eqb[:], eqb[:], 1e30, None,
                                op0=mybir.AluOpType.mult)
        tmp = spool.tile([1, F2], F32, name="tmp")
        nc.vector.tensor_tensor(tmp[:], sc_sb[:], eqb[:],
                                op=mybir.AluOpType.subtract)
        m2 = spool.tile([1, NL * S], F32, name="m2")
        nc.vector.reduce_max(m2[:], seg(tmp[:]), axis=X)
        maskrow = spool.tile([1, F2], F32, name="maskrow")
        nc.vector.tensor_tensor(v4(maskrow[:]), v4(sc_sb[:]), bc(m2[:]), op=ge)

        # broadcast mask to all 128 partitions scaled by 1/512 (the 1/2 top-k
        # mean * 1/256 stripe mean): (1/512)ones(1,128).T @ maskrow(1,480)
        psb = ppool.tile([128, F2], F32, name="psb", tag="psb")
        nc.tensor.matmul(psb[:], ones_row[:], maskrow[:], start=True, stop=True)

        for ci in range(2):
            prod = spool.tile([128, F2], F32, name=f"prod{ci}", tag=f"prod{ci}")
            nc.vector.tensor_tensor(
                prod[:], ffq[ci][:].rearrange("p a b c -> p (a b c)"),
                psb[:], op=mybir.AluOpType.mult)
            red = spool.tile([128, NL * S], F32, name=f"red{ci}", tag=f"red{ci}")
            nc.vector.reduce_sum(
                red[:], prod[:].rearrange("p (n t s) -> p n s t", n=NL, t=T, s=S),
                axis=X,
            )
            for n_ in range(NL):
                nc.sync.dma_start(
                    out[n_, ci * 1024:(ci + 1) * 1024]
                    .rearrange("(p s) -> p s", p=128),
                    red[:, n_ * S:(n_ + 1) * S],
                )

    nc.compile()
    return nc


_NC_CACHE: list = []


def _get_program() -> bacc.Bacc:
    if not _NC_CACHE:
        _NC_CACHE.append(build_program())
    return _NC_CACHE[0]


def _cast_core(xf: np.ndarray, i: int) -> np.ndarray:
    return np.ascontiguousarray(
        xf[i * NL:(i + 1) * NL].reshape(NL, C, T, HW), dtype=np.float16)


def kernel(x: np.ndarray) -> np.ndarray:
    assert x.shape == (N, C, T, H, W), x.shape
    nc = _get_program()
    xf = np.asarray(x)
    with ThreadPoolExecutor(N_CORES) as ex:
        cores = list(ex.map(lambda i: _cast_core(xf, i), range(N_CORES)))
    in_maps = [{"x": cores[i]} for i in range(N_CORES)]
    res = run_bass_kernel_spmd(nc, in_maps, core_ids=list(range(N_CORES)))
    parts = [res.results[i]["out"] for i in range(N_CORES)]
    raw = np.concatenate(parts, axis=0)  # (16, 2048), col = ci*1024 + cl*8 + s
    # reorder columns to the reference's s*256 + (ci*128 + cl)
    full = raw.reshape(N, 2, 128, S).transpose(0, 3, 1, 2).reshape(N, OUT_COLS)
    return np.ascontiguousarray(full)


# revision 10
# speedup vs baseline: 1.1443x; 1.1443x over previous
"""Trainium2 Bass kernel for LocalSpatioTemporalPooling (topk masking).

Reference computation (per sample n):
  x: (N=16, C=256, T=30, H=64, W=32) f32
  ff[n,c,t,s]   = mean over the (8,32) stripe s of the (H,W) plane
  score[n,t,s]  = sum_c ff^2   (monotone in the reference's sqrt/clip score)
  top-2 t per (n,s) by score; output[n, s*256+c] = mean of ff over those 2 t.

Strategy: pure data parallel over batch N across 8 cores (2 samples/core).
Inputs are shipped to the device as fp16 (exact top-2 sets and ~2e-4 output
rel err for randn inputs -- verified against the f32 reference), which halves
both host->device traffic and the on-device HBM read volume.

Per core (62.9 MB fp16):
  phase 1 (memory bound): stream x in 20 tiles of 3 MB, partition = channel
    (128 channels per half ci), free = 6 frames x 2048 spatial.  GpSimd folds
    element pairs within each 256-element stripe segment (fp16 add), DVE then
    reduce_sums the folded 128-element segments into f32 stripe sums, laid
    out as ffq[ci][c_part, (n, t, s)] directly in SBUF -- no DRAM roundtrip.
  phase 2 (tiny): square + ones-matmul on PE -> per-(n,t,s) score; top-2 via
    mask = (score >= 2nd max) scaled by 1/512 (fuses the /2 top-k mean and
    the /256 stripe mean); broadcast mask over partitions via PE outer
    product; masked reduce over t -> out columns (ci, c_local, s), reordered
    on host.
"""

import sys
from concurrent.futures import ThreadPoolExecutor
from contextlib import ExitStack

for _p in ("/opt/trn_rl_repo",):
    if _p not in sys.path:
        sys.path.insert(0, _p)

import numpy as np

import concourse.bass as bass
import concourse.tile as tile
from concourse import bacc, mybir
from concourse.bass_utils import run_bass_kernel_spmd

N_CORES = 8
N, C, T, H, W = 16, 256, 30, 64, 32
S = 8            # stripes
E = (H // S) * W  # 256 elements per stripe segment
NL = N // N_CORES  # samples per core = 2
HW = H * W       # 2048
T_SUB = 6        # frames per tile
NT = T // T_SUB  # 5 t-chunks
TS = T * S       # 240
F2 = NL * TS     # 480
OUT_COLS = S * C  # 2048
F32 = mybir.dt.float32
F16 = mybir.dt.float16
X = mybir.AxisListType.X


def build_program() -> bacc.Bacc:
    nc = bacc.Bacc("TRN2", target_bir_lowering=False, debug=False,
                   num_devices=N_CORES)
    x = nc.dram_tensor("x", [NL, C, T, HW], F16, kind="ExternalInput").ap()
    out = nc.dram_tensor("out", [NL, OUT_COLS], F32, kind="ExternalOutput").ap()

    with tile.TileContext(nc) as tc, ExitStack() as ctx:
        xpool = ctx.enter_context(tc.tile_pool(name="xtiles", bufs=4))
        fpool = ctx.enter_context(tc.tile_pool(name="folds", bufs=3))
        cpool = ctx.enter_context(tc.tile_pool(name="consts", bufs=1))
        spool = ctx.enter_context(tc.tile_pool(name="small", bufs=1))
        ppool = ctx.enter_context(tc.tile_pool(name="psum", bufs=1, space="PSUM"))

        # ---- phase 1: per-stripe sums -> ffq[ci][c, (n, t, s)] in SBUF ----
        # Per tile: DVE folds the halves of each 256-el stripe segment (fp16
        # add; 128-el runs keep the 2x packed mode, ~3.4 us), GpSimd folds
        # once more (64-el runs, ~7.4 us -- GpSimd has no packing modes to
        # lose), DVE reduce_sums the 64-el segments into f32 (~3.3 us).  No
        # stage exceeds the ~9.6 us/tile DMA cadence -> DMA-bound stream.
        # The last TAIL_TILES tiles skip the GpSimd hop (DVE reduces the
        # once-folded tile directly) to shorten the post-DMA tail.
        # The reduce for tile i is emitted after fold1 of tile i+LAG: engine
        # queues execute in order, so an immediately-emitted reduce (waiting
        # on GpSimd's fold2) would stall the next fold1 behind it on DVE.
        add = mybir.AluOpType.add
        NTILES = NL * 2 * NT
        TAIL_TILES = 2
        LAG = 2
        ffq = [cpool.tile([128, NL, T, S], F32, name=f"ffq{ci}") for ci in range(2)]
        pending = []
        i = 0
        for ci in range(2):
            for n_ in range(NL):
                for tc_ in range(NT):
                    xt = xpool.tile([128, T_SUB * HW], F16, name="xt", tag="xt")
                    eng = nc.sync if (i % 2 == 0) else nc.scalar
                    eng.dma_start(
                        xt[:],
                        x[n_, ci * 128:(ci + 1) * 128,
                          tc_ * T_SUB:(tc_ + 1) * T_SUB, :]
                        .rearrange("c k m -> c (k m)"),
                    )
                    v4 = xt[:].rearrange("p (k s e) -> p k s e", k=T_SUB, s=S)
                    ft = fpool.tile([128, T_SUB, S, E // 2], F16,
                                    name="ft", tag="ft")
                    nc.vector.tensor_tensor(
                        ft[:], v4[:, :, :, 0:E // 2], v4[:, :, :, E // 2:E],
                        op=add,
                    )
                    out_slice = ffq[ci][:, n_, tc_ * T_SUB:(tc_ + 1) * T_SUB, :]
                    if i < NTILES - TAIL_TILES:
                        ft2 = fpool.tile([128, T_SUB, S, E // 4], F16,
                                         name="ft2", tag="ft2")
                        nc.gpsimd.tensor_tensor(
                            ft2[:], ft[:, :, :, 0:E // 4],
                            ft[:, :, :, E // 4:E // 2], op=add,
                        )
                        pending.append((out_slice, ft2))
                    else:
                        pending.append((out_slice, ft))
                    if len(pending) > LAG:
                        osl, src = pending.pop(0)
                        nc.vector.reduce_sum(osl, src[:], axis=X)
                    i += 1
        for osl, src in pending:
            nc.vector.reduce_sum(osl, src[:], axis=X)

        # ---- phase 2: scores, top-2 mask, masked mean ----
        ones_col = cpool.tile([128, 1], F32)   # K=128 stationary: column of ones
        nc.vector.memset(ones_col[:], 1.0)
        ones_row = cpool.tile([1, 128], F32)   # K=1 stationary: row of 1/512
        nc.vector.memset(ones_row[:], 1.0 / 512.0)

        psc = ppool.tile([1, F2], F32, name="psc", tag="psc")  # sum_c ff^2
        for ci in range(2):
            sq = spool.tile([128, F2], F32, name=f"sq{ci}", tag=f"sq{ci}")
            nc.scalar.square(sq[:], ffq[ci][:].rearrange("p a b c -> p (a b c)"))
            nc.tensor.matmul(psc[:], ones_col[:], sq[:],
                             start=(ci == 0), stop=(ci == 1))

        sc_sb = spool.tile([1, F2], F32, name="sc_sb")
        nc.scalar.copy(sc_sb[:], psc[:])

        # top-2 mask per (n, s) segment, computed in place on one partition.
        # seg: (q, n, s, t) view for per-(n,s) reduces over t;
        # v4/bc: matched 4D (q, n, t, s) iteration, bc has stride-0 over t.
        def seg(ap):
            return ap.rearrange("q (n t s) -> q n s t", n=NL, t=T, s=S)

        def v4(ap):
            return ap.rearrange("q (n t s) -> q n t s", n=NL, t=T, s=S)

        def bc(ap):
            return (ap.rearrange("q (n s) -> q n s", n=NL)[:, :, None, :]
                    .broadcast_to((1, NL, T, S)))

        ge = mybir.AluOpType.is_ge
        m1 = spool.tile([1, NL * S], F32, name="m1")
        nc.vector.reduce_max(m1[:], seg(sc_sb[:]), axis=X)
        eqb = spool.tile([1, F2], F32, name="eqb")
        nc.vector.tensor_tensor(v4(eqb[:]), v4(sc_sb[:]), bc(m1[:]), op=ge)
        nc.vector.tensor_scalar(eqb[:], eqb[:], 1e30, None,
                                op0=mybir.AluOpType.mult)
        tmp = spool.tile([1, F2], F32, name="tmp")
        nc.vector.tensor_tensor(tmp[:], sc_sb[:], eqb[:],
                                op=mybir.AluOpType.subtract)
        m2 = spool.tile([1, NL * S], F32, name="m2")
        nc.vector.reduce_max(m2[:], seg(tmp[:]), axis=X)
        maskrow = spool.tile([1, F2], F32, name="maskrow")
        nc.vector.tensor_tensor(v4(maskrow[:]), v4(sc_sb[:]), bc(m2[:]), op=ge)

        # broadcast mask to all 128 partitions scaled by 1/512 (the 1/2 top-k
        # mean * 1/256 stripe mean): (1/512)ones(1,128).T @ maskrow(1,480)
        psb = ppool.tile([128, F2], F32, name="psb", tag="psb")
        nc.tensor.matmul(psb[:], ones_row[:], maskrow[:], start=True, stop=True)

        for ci in range(2):
            prod = spool.tile([128, F2], F32, name=f"prod{ci}", tag=f"prod{ci}")
            nc.vector.tensor_tensor(
                prod[:], ffq[ci][:].rearrange("p a b c -> p (a b c)"),
                psb[:], op=mybir.AluOpType.mult)
            red = spool.tile([128, NL * S], F32, name=f"red{ci}", tag=f"red{ci}")
            nc.vector.reduce_sum(
                red[:], prod[:].rearrange("p (n t s) -> p n s t", n=NL, t=T, s=S),
                axis=X,
            )
            for n_ in range(NL):
                nc.sync.dma_start(
                    out[n_, ci * 1024:(ci + 1) * 1024]
                    .rearrange("(p s) -> p s", p=128),
                    red[:, n_ * S:(n_ + 1) * S],
                )

    nc.compile()
    return nc


_NC_CACHE: list = []


def _get_program() -> bacc.Bacc:
    if not _NC_CACHE:
        _NC_CACHE.append(build_program())
    return _NC_CACHE[0]


def _cast_core(xf: np.ndarray, i: int) -> np.ndarray:
    return np.ascontiguousarray(
        xf[i * NL:(i + 1) * NL].reshape(NL, C, T, HW), dtype=np.float16)


def kernel(x: np.ndarray) -> np.ndarray:
    assert x.shape == (N, C, T, H, W), x.shape
    nc = _get_program()
    xf = np.asarray(x)
    with ThreadPoolExecutor(N_CORES) as ex:
        cores = list(ex.map(lambda i: _cast_core(xf, i), range(N_CORES)))
    in_maps = [{"x": cores[i]} for i in range(N_CORES)]
    res = run_bass_kernel_spmd(nc, in_maps, core_ids=list(range(N_CORES)))
    parts = [res.results[i]["out"] for i in range(N_CORES)]
    raw = np.concatenate(parts, axis=0)  # (16, 2048), col = ci*1024 + cl*8 + s
    # reorder columns to the reference's s*256 + (ci*128 + cl)
    full = raw.reshape(N, 2, 128, S).transpose(0, 3, 1, 2).reshape(N, OUT_COLS)
    return np.ascontiguousarray(full)
